# revision 8
# baseline (speedup 1.0000x reference)
"""Complex-magnitude MaxPool2d (k=2, s=2) Trainium2 Bass kernel.

Input  x:  [16, 2, 64, 224, 224] f32  (plane 0 = real, plane 1 = imag)
Output:    [16, 2, 64, 112, 112] f32  (value of the window element with the
                                       largest |z|^2 = re^2 + im^2)

Sharding: pure data parallel over batch: 16 / 8 cores = 2 examples per core.
Per core the 2(batch) x 64(channel) = 128 image planes map 1:1 onto the 128
SBUF partitions; each 224x224 plane is streamed in chunks of 14 rows.

Selection is an argmax tournament that reproduces jnp.argmax's first-index
tie-break exactly: horizontal pass first (left/even column wins ties via
is_ge), then vertical (top row wins ties). norm^2 is fl(fl(re*re)+fl(im*im))
in f32, the same rounding as the reference expression.

The selects are done *in place* with copy_predicated: the odd-column slots
of the input tiles already hold the select's false branch, so the horizontal
winner overwrites them directly; the vertical winner then overwrites the
bottom-row/odd-column slots, which are finally compacted and DMA'd out.
Buffer reuse: norm2 is accumulated in place over sq_re; the horizontal mask
lives in sq_im's even slots, the horizontal max in norm2's odd slots, and
the vertical mask in sq_im's odd slots.
"""

import numpy as np

import concourse.bass as bass
import concourse.mybir as mybir
from concourse import bacc, bass_utils, tile

# Per-core shard geometry (hardcoded; kernel.py must be self-contained).
NCORES = 8
B = 2            # batch per core
RI = 2           # real/imag planes
C = 64           # channels
H = W = 224
HO, WO = H // 2, W // 2
P = 128          # SBUF partitions = B * C
R = 14           # image rows per chunk
NCHUNK = H // R  # 16

F32 = mybir.dt.float32
OP = mybir.AluOpType

_NC_CACHE = []


def _v4(ap):
    # [P, R*W] tile -> [P, r, w, t]: row r, column-pair w, parity t
    return ap.rearrange("p (r w t) -> p r w t", r=R, w=WO, t=2)


def _v5(ap):
    # [P, R*W] tile -> [P, rp, rt, w, t]: row-pair rp, row parity rt
    return ap.rearrange("p (rp rt w t) -> p rp rt w t", rp=R // 2, rt=2, w=WO, t=2)


# how many of the R rows of sq_re GPSIMD computes (engine balance);
# the DVE covers the rest.  GPSIMD also computes all of sq_im.
GPS_SQRE_ROWS = 12


def _build_nc() -> bass.Bass:
    nc = bacc.Bacc("TRN2", target_bir_lowering=False, debug=False)
    x = nc.dram_tensor("x", [B, RI, C, H, W], F32, kind="ExternalInput").ap()
    out = nc.dram_tensor("out", [B, RI, C, HO, WO], F32, kind="ExternalOutput").ap()

    gs = GPS_SQRE_ROWS * W  # gpsimd share of sq_re, in elements

    with tile.TileContext(nc) as tc:
        with tc.tile_pool(name="pool", bufs=3) as pool:
            for k in range(NCHUNK):
                r0 = k * R
                xre = pool.tile([P, R * W], F32, tag="xre")
                xim = pool.tile([P, R * W], F32, tag="xim")
                for b in range(B):
                    nc.sync.dma_start(
                        out=xre[b * C : (b + 1) * C, :], in_=x[b, 0, :, r0 : r0 + R, :]
                    )
                    nc.sync.dma_start(
                        out=xim[b * C : (b + 1) * C, :], in_=x[b, 1, :, r0 : r0 + R, :]
                    )

                # norm2 = re*re + im*im (f32, reference rounding); nrm == sqre
                sqre = pool.tile([P, R * W], F32, tag="sqre")
                sqim = pool.tile([P, R * W], F32, tag="sqim")
                nc.gpsimd.tensor_tensor(out=sqim[:], in0=xim[:], in1=xim[:], op=OP.mult)
                if gs:
                    nc.gpsimd.tensor_tensor(
                        out=sqre[:, :gs], in0=xre[:, :gs], in1=xre[:, :gs], op=OP.mult
                    )
                if gs < R * W:
                    nc.vector.tensor_tensor(
                        out=sqre[:, gs:], in0=xre[:, gs:], in1=xre[:, gs:], op=OP.mult
                    )
                nc.vector.tensor_tensor(out=sqre[:], in0=sqre[:], in1=sqim[:], op=OP.add)

                nrm4, sqim4 = _v4(sqre), _v4(sqim)
                xre4, xim4 = _v4(xre), _v4(xim)
                nE, nO = nrm4[:, :, :, 0], nrm4[:, :, :, 1]
                # horizontal mask (int32 1/0): even/left wins ties
                cH = sqim4[:, :, :, 0].bitcast(mybir.dt.int32)
                nc.vector.tensor_tensor(out=cH, in0=nE, in1=nO, op=OP.is_ge)
                # horizontal max -> nrm odd slots (in place)
                nc.vector.tensor_tensor(out=nO, in0=nE, in1=nO, op=OP.max)
                # select: odd slots already hold the right/odd value
                nc.vector.copy_predicated(
                    out=xre4[:, :, :, 1], mask=cH, data=xre4[:, :, :, 0]
                )
                nc.vector.copy_predicated(
                    out=xim4[:, :, :, 1], mask=cH, data=xim4[:, :, :, 0]
                )

                # vertical pass over row pairs; mH lives in nrm odd slots
                nrm5, sqim5 = _v5(sqre), _v5(sqim)
                xre5, xim5 = _v5(xre), _v5(xim)
                # vertical mask (int32 1/0): top wins ties
                cV = sqim5[:, :, 0, :, 1].bitcast(mybir.dt.int32)
                nc.vector.tensor_tensor(
                    out=cV, in0=nrm5[:, :, 0, :, 1], in1=nrm5[:, :, 1, :, 1],
                    op=OP.is_ge,
                )
                nc.vector.copy_predicated(
                    out=xre5[:, :, 1, :, 1], mask=cV, data=xre5[:, :, 0, :, 1]
                )
                nc.vector.copy_predicated(
                    out=xim5[:, :, 1, :, 1], mask=cV, data=xim5[:, :, 0, :, 1]
                )

                # compact the pooled values and store
                outre = pool.tile([P, (R // 2) * WO], F32, tag="outre")
                outim = pool.tile([P, (R // 2) * WO], F32, tag="outim")
                nc.scalar.copy(out=outre[:], in_=xre5[:, :, 1, :, 1])
                nc.scalar.copy(out=outim[:], in_=xim5[:, :, 1, :, 1])

                o0, o1 = r0 // 2, (r0 + R) // 2
                for b in range(B):
                    nc.sync.dma_start(
                        out=out[b, 0, :, o0:o1, :],
                        in_=outre[b * C : (b + 1) * C, :],
                    )
                    nc.sync.dma_start(
                        out=out[b, 1, :, o0:o1, :],
                        in_=outim[b * C : (b + 1) * C, :],
                    )
    nc.compile()
    return nc


def get_nc() -> bass.Bass:
    if not _NC_CACHE:
        _NC_CACHE.append(_build_nc())
    return _NC_CACHE[0]


def kernel(x: np.ndarray, **run_kwargs) -> np.ndarray:
    nc = get_nc()
    xs = np.asarray(x, dtype=np.float32)
    assert xs.shape == (NCORES * B, RI, C, H, W), xs.shape
    in_maps = [{"x": xs[B * i : B * (i + 1)]} for i in range(NCORES)]
    res = bass_utils.run_bass_kernel_spmd(
        nc, in_maps, core_ids=list(range(NCORES)), **run_kwargs
    )
    out = np.concatenate([res.results[i]["out"] for i in range(NCORES)], axis=0)
    if run_kwargs:
        kernel.last_results = res
    return out


# revision 9
# speedup vs baseline: 1.1504x; 1.1504x over previous
"""Complex-magnitude MaxPool2d (k=2, s=2) Trainium2 Bass kernel.

Input  x:  [16, 2, 64, 224, 224] f32  (plane 0 = real, plane 1 = imag)
Output:    [16, 2, 64, 112, 112] f32  (value of the window element with the
                                       largest |z|^2 = re^2 + im^2)

Sharding: pure data parallel over batch: 16 / 8 cores = 2 examples per core.
Per core the 2(batch) x 64(channel) = 128 image planes map 1:1 onto the 128
SBUF partitions; each 224x224 plane is streamed in chunks of 14 rows.

Selection reproduces jnp.argmax's first-index tie-break exactly:
horizontal pass first (left/even column wins ties via is_ge), then vertical
(top row wins ties).  norm2 = fl(fl(re*re)+fl(im*im)) in f32 — ACT's Square
activation and GPSIMD's f32 add are bit-exact with the reference expression
(hardware-verified), so selections match the reference everywhere,
including exact ties.

Engine split (measured rates):
  ScalarE : squares (one ACT op per chunk), horizontal pre-fill copy
  GPSIMD  : norm add (in place over the squares), vertical pre-fill copy
  VectorE : is_ge masks + copy_predicated selects.  Masks and predicated
            dst are kept contiguous (2x faster than strided), and each
            pred selects re+im together via a step-0 broadcast mask.
  DMA     : ~64 MB/core at ~360 GB/s is the roofline.
"""

import numpy as np

import concourse.bass as bass
import concourse.mybir as mybir
from concourse import bacc, bass_utils, tile

# Per-core shard geometry (hardcoded; kernel.py must be self-contained).
NCORES = 8
B = 2            # batch per core
RI = 2           # real/imag planes
C = 64           # channels
H = W = 224
HO, WO = H // 2, W // 2
P = 128          # SBUF partitions = B * C
R = 14           # image rows per chunk
NCHUNK = H // R  # 16
N = R * W        # free elements per plane per chunk (3136)

F32 = mybir.dt.float32
I8 = mybir.dt.uint8
OP = mybir.AluOpType
ACTF = mybir.ActivationFunctionType

_NC_CACHE = []


def _build_nc() -> bass.Bass:
    nc = bacc.Bacc("TRN2", target_bir_lowering=False, debug=False)
    x = nc.dram_tensor("x", [B, RI, C, H, W], F32, kind="ExternalInput").ap()
    out = nc.dram_tensor("out", [B, RI, C, HO, WO], F32, kind="ExternalOutput").ap()

    with tile.TileContext(nc) as tc:
        with tc.tile_pool(name="pool", bufs=2) as pool:
            for k in range(NCHUNK):
                r0 = k * R
                # xri: [re block | im block], each N elems per partition
                xri = pool.tile([P, 2 * N], F32, tag="xri")
                for b in range(B):
                    for ri in range(RI):
                        nc.sync.dma_start(
                            out=xri[b * C : (b + 1) * C, ri * N : (ri + 1) * N],
                            in_=x[b, ri, :, r0 : r0 + R, :],
                        )

                # squares of everything in one ACT op; then norm2 in place
                # over the re half (nrm == sqri[:, :N])
                sqri = pool.tile([P, 2 * N], F32, tag="sqri")
                nc.scalar.activation(out=sqri[:], in_=xri[:], func=ACTF.Square)
                nrm = sqri[:, :N]
                nc.gpsimd.tensor_tensor(
                    out=nrm, in0=nrm, in1=sqri[:, N:], op=OP.add
                )

                nrm4 = nrm.rearrange("p (r w t) -> p r w t", r=R, w=WO, t=2)
                nE, nO = nrm4[:, :, :, 0], nrm4[:, :, :, 1]

                # horizontal mask (contiguous u8): even/left wins ties
                cH = pool.tile([P, R * WO], I8, tag="cH")
                cH3 = cH.rearrange("p (r w) -> p r w", r=R, w=WO)
                nc.vector.tensor_tensor(out=cH3, in0=nE, in1=nO, op=OP.is_ge)
                # horizontal norm max -> nrm odd slots (in place)
                nc.vector.tensor_tensor(out=nO, in0=nE, in1=nO, op=OP.max)

                # horizontal select of (re, im) together into a compact tile:
                # pre-fill with the odd/right value, overwrite where cH
                xri5 = xri.rearrange(
                    "p (pl r w t) -> p pl r w t", pl=RI, r=R, w=WO, t=2
                )
                riH = pool.tile([P, RI * R * WO], F32, tag="riH")
                riH4 = riH.rearrange("p (pl r w) -> p pl r w", pl=RI, r=R, w=WO)
                nc.scalar.copy(out=riH4, in_=xri5[:, :, :, :, 1])
                cHb = (
                    cH3.unsqueeze(1).broadcast_to([P, RI, R, WO])
                )
                nc.vector.copy_predicated(
                    out=riH4, mask=cHb, data=xri5[:, :, :, :, 0]
                )

                # vertical mask from the horizontal maxes: top wins ties
                nrm5 = nrm.rearrange(
                    "p (rp rt w t) -> p rp rt w t", rp=R // 2, rt=2, w=WO, t=2
                )
                cV = pool.tile([P, (R // 2) * WO], I8, tag="cV")
                cV3 = cV.rearrange("p (rp w) -> p rp w", rp=R // 2, w=WO)
                nc.vector.tensor_tensor(
                    out=cV3, in0=nrm5[:, :, 0, :, 1], in1=nrm5[:, :, 1, :, 1],
                    op=OP.is_ge,
                )

                # vertical select into the output staging tile
                riH5 = riH.rearrange(
                    "p (pl rp rt w) -> p pl rp rt w", pl=RI, rp=R // 2, rt=2, w=WO
                )
                outri = pool.tile([P, RI * (R // 2) * WO], F32, tag="outri")
                outri4 = outri.rearrange(
                    "p (pl rp w) -> p pl rp w", pl=RI, rp=R // 2, w=WO
                )
                nc.gpsimd.tensor_copy(out=outri4, in_=riH5[:, :, :, 1, :])
                cVb = cV3.unsqueeze(1).broadcast_to([P, RI, R // 2, WO])
                nc.vector.copy_predicated(
                    out=outri4, mask=cVb, data=riH5[:, :, :, 0, :]
                )

                o0, o1 = r0 // 2, (r0 + R) // 2
                hw = (R // 2) * WO
                for b in range(B):
                    for ri in range(RI):
                        nc.sync.dma_start(
                            out=out[b, ri, :, o0:o1, :],
                            in_=outri[b * C : (b + 1) * C, ri * hw : (ri + 1) * hw],
                        )
    nc.compile()
    return nc


def get_nc() -> bass.Bass:
    if not _NC_CACHE:
        _NC_CACHE.append(_build_nc())
    return _NC_CACHE[0]


def kernel(x: np.ndarray, **run_kwargs) -> np.ndarray:
    nc = get_nc()
    xs = np.asarray(x, dtype=np.float32)
    assert xs.shape == (NCORES * B, RI, C, H, W), xs.shape
    in_maps = [{"x": xs[B * i : B * (i + 1)]} for i in range(NCORES)]
    res = bass_utils.run_bass_kernel_spmd(
        nc, in_maps, core_ids=list(range(NCORES)), **run_kwargs
    )
    out = np.concatenate([res.results[i]["out"] for i in range(NCORES)], axis=0)
    if run_kwargs:
        kernel.last_results = res
    return out


# revision 11
# speedup vs baseline: 1.2940x; 1.1248x over previous
"""Complex-magnitude MaxPool2d (k=2, s=2) Trainium2 Bass kernel.

Input  x:  [16, 2, 64, 224, 224] f32  (plane 0 = real, plane 1 = imag)
Output:    [16, 2, 64, 112, 112] f32  (value of the window element with the
                                       largest |z|^2 = re^2 + im^2)

Sharding: pure data parallel over batch: 16 / 8 cores = 2 examples per core.
Per core the 2(batch) x 64(channel) = 128 image planes map 1:1 onto the 128
SBUF partitions; each 224x224 plane is streamed in chunks of 14 rows.

Selection reproduces jnp.argmax's first-index tie-break exactly:
horizontal pass first (left/even column wins ties via is_ge), then vertical
(top row wins ties).  norm2 = fl(fl(re*re)+fl(im*im)) in f32 — ACT's Square
activation and GPSIMD's f32 add are bit-exact with the reference expression
(hardware-verified), so selections match the reference everywhere,
including exact ties.

Engine split (measured rates):
  ScalarE : squares (one ACT op per chunk), horizontal pre-fill copy
  GPSIMD  : norm add (in place over the squares), vertical pre-fill copy
  VectorE : is_ge masks + copy_predicated selects.  Masks and predicated
            dst are kept contiguous (2x faster than strided), and each
            pred selects re+im together via a step-0 broadcast mask.
  DMA     : ~64 MB/core at ~360 GB/s is the roofline.
"""

import numpy as np

import concourse.bass as bass
import concourse.mybir as mybir
from concourse import bacc, bass_utils, tile

# Per-core shard geometry (hardcoded; kernel.py must be self-contained).
NCORES = 8
B = 2            # batch per core
RI = 2           # real/imag planes
C = 64           # channels
H = W = 224
HO, WO = H // 2, W // 2
P = 128          # SBUF partitions = B * C
R = 14           # image rows per chunk
NCHUNK = H // R  # 16
N = R * W        # free elements per plane per chunk (3136)

F32 = mybir.dt.float32
I8 = mybir.dt.uint8
OP = mybir.AluOpType
ACTF = mybir.ActivationFunctionType

_NC_CACHE = []


def _build_nc() -> bass.Bass:
    nc = bacc.Bacc("TRN2", target_bir_lowering=False, debug=False)
    x = nc.dram_tensor("x", [B, RI, C, H, W], F32, kind="ExternalInput").ap()
    out = nc.dram_tensor("out", [B, RI, C, HO, WO], F32, kind="ExternalOutput").ap()

    # output staging: GROUP chunks of pooled rows accumulate in SBUF so each
    # store DMA moves a long contiguous run per partition (DMA efficiency)
    GROUP = 4
    SROWS = GROUP * (R // 2)  # 28 output rows per staged store

    with tile.TileContext(nc) as tc:
        with tc.tile_pool(name="pool", bufs=2) as pool:
            stage = None
            for k in range(NCHUNK):
                r0 = k * R
                # xri: [re block | im block], each N elems per partition
                xri = pool.tile([P, 2 * N], F32, tag="xri")
                for b in range(B):
                    for ri in range(RI):
                        nc.sync.dma_start(
                            out=xri[b * C : (b + 1) * C, ri * N : (ri + 1) * N],
                            in_=x[b, ri, :, r0 : r0 + R, :],
                        )

                # squares of everything in one ACT op; then norm2 in place
                # over the re half (nrm == sqri[:, :N])
                sqri = pool.tile([P, 2 * N], F32, tag="sqri")
                nc.scalar.activation(out=sqri[:], in_=xri[:], func=ACTF.Square)
                nrm = sqri[:, :N]
                nc.gpsimd.tensor_tensor(
                    out=nrm, in0=nrm, in1=sqri[:, N:], op=OP.add
                )

                nrm4 = nrm.rearrange("p (r w t) -> p r w t", r=R, w=WO, t=2)
                nE, nO = nrm4[:, :, :, 0], nrm4[:, :, :, 1]

                # horizontal mask (contiguous u8): even/left wins ties
                cH = pool.tile([P, R * WO], I8, tag="cH")
                cH3 = cH.rearrange("p (r w) -> p r w", r=R, w=WO)
                nc.vector.tensor_tensor(out=cH3, in0=nE, in1=nO, op=OP.is_ge)
                # horizontal norm max -> nrm odd slots (in place)
                nc.vector.tensor_tensor(out=nO, in0=nE, in1=nO, op=OP.max)

                # horizontal select of (re, im) together into a compact tile:
                # pre-fill with the odd/right value, overwrite where cH
                xri5 = xri.rearrange(
                    "p (pl r w t) -> p pl r w t", pl=RI, r=R, w=WO, t=2
                )
                riH = pool.tile([P, RI * R * WO], F32, tag="riH")
                riH4 = riH.rearrange("p (pl r w) -> p pl r w", pl=RI, r=R, w=WO)
                nc.scalar.copy(out=riH4, in_=xri5[:, :, :, :, 1])
                cHb = (
                    cH3.unsqueeze(1).broadcast_to([P, RI, R, WO])
                )
                nc.vector.copy_predicated(
                    out=riH4, mask=cHb, data=xri5[:, :, :, :, 0]
                )

                # vertical mask from the horizontal maxes: top wins ties
                nrm5 = nrm.rearrange(
                    "p (rp rt w t) -> p rp rt w t", rp=R // 2, rt=2, w=WO, t=2
                )
                cV = pool.tile([P, (R // 2) * WO], I8, tag="cV")
                cV3 = cV.rearrange("p (rp w) -> p rp w", rp=R // 2, w=WO)
                nc.vector.tensor_tensor(
                    out=cV3, in0=nrm5[:, :, 0, :, 1], in1=nrm5[:, :, 1, :, 1],
                    op=OP.is_ge,
                )

                # vertical select into the staged output tile (kept across
                # GROUP chunks so the store DMA moves long contiguous runs)
                riH5 = riH.rearrange(
                    "p (pl rp rt w) -> p pl rp rt w", pl=RI, rp=R // 2, rt=2, w=WO
                )
                if k % GROUP == 0:
                    stage = pool.tile([P, RI * SROWS * WO], F32, tag="stage")
                stage4 = stage.rearrange(
                    "p (pl r w) -> p pl r w", pl=RI, r=SROWS, w=WO
                )
                s0 = (k % GROUP) * (R // 2)
                dst = stage4[:, :, s0 : s0 + R // 2, :]
                nc.scalar.copy(out=dst, in_=riH5[:, :, :, 1, :])
                cVb = cV3.unsqueeze(1).broadcast_to([P, RI, R // 2, WO])
                nc.vector.copy_predicated(
                    out=dst, mask=cVb, data=riH5[:, :, :, 0, :]
                )

                if (k + 1) % GROUP == 0:
                    g0 = (k + 1 - GROUP) * (R // 2)
                    hw = SROWS * WO
                    for b in range(B):
                        for ri in range(RI):
                            nc.sync.dma_start(
                                out=out[b, ri, :, g0 : g0 + SROWS, :],
                                in_=stage[b * C : (b + 1) * C, ri * hw : (ri + 1) * hw],
                            )
    nc.compile()
    return nc


def get_nc() -> bass.Bass:
    if not _NC_CACHE:
        _NC_CACHE.append(_build_nc())
    return _NC_CACHE[0]


def kernel(x: np.ndarray, **run_kwargs) -> np.ndarray:
    nc = get_nc()
    xs = np.asarray(x, dtype=np.float32)
    assert xs.shape == (NCORES * B, RI, C, H, W), xs.shape
    in_maps = [{"x": xs[B * i : B * (i + 1)]} for i in range(NCORES)]
    res = bass_utils.run_bass_kernel_spmd(
        nc, in_maps, core_ids=list(range(NCORES)), **run_kwargs
    )
    out = np.concatenate([res.results[i]["out"] for i in range(NCORES)], axis=0)
    if run_kwargs:
        kernel.last_results = res
    return out
